# revision 26
# baseline (speedup 1.0000x reference)
"""AttentionProtoNet pooling kernel for 8x TRN2 NeuronCores.

reference (per sample of B=64, L=512, H=768):
    upsilon = tanh(hs @ W_fc.T + b_fc)        [L, H]
    nu      = upsilon @ W_nu                  [L]
    alphas  = softmax(nu)                     [L]
    pooled  = alphas @ hs                     [H]

Strategy: data-parallel over B (8 samples per core). The big GEMM runs in
bf16 (1 cycle/row on the PE at the full 2.4 GHz pstate) against a single
bf16 X^T copy that also feeds the pooling stage. Output channels are
sorted by |W_nu| on the host: upsilon only exists to produce the scalar
nu = W_nu . tanh(...), so the 512 lowest-|W_nu| channels can round their
tanh output to fp8e4 with negligible effect, letting the nu contraction
run as two fp8 DoubleRow matmuls (256-deep, 0.5 cyc/row) plus two bf16
ones - half the PE cost of a pure bf16 nu. W_nu rides along x64 (fp8
needs the scale to stay normal; exp() folds 1/64 back in for free).
Weights + biases + W_nu ship as ONE contiguous-per-partition DMA (small
strided lines run at ~30 GB/s vs ~170 GB/s for 8KB lines); X samples are
monolithic per-sample transfers split across the gpsimd/scalar queues.
tanh runs on ACT straight out of PSUM (per-partition bias), softmax on 1
partition with bf16 exp, alphas broadcast via GpSimd, weighted-sum
pooling on the VectorEngine in bf16, outputs drain per-sample through a
tiny PE transpose. Sample s's tail (nu/softmax/pool) is interleaved into
sample s+1's GEMM block so the PE stream stays dense.
"""

import sys

sys.path.insert(0, "/opt/trn_rl_repo")

import numpy as np
import ml_dtypes

B, L, H = 64, 512, 768
NCORES = 8
SPC = B // NCORES            # samples per core
HC = H // 128                # 128-partition chunks of H
N8C = 4                      # ups chunks (lowest |W_nu|) rounded to fp8
NUM = 8                      # wnu occupies m=0 of each 16B-strided plane
CW = HC + HC + N8C * 16 // 2   # head cols: bias | wnu*64 bf16 | packed fp8
WARMUP_MM = 22               # junk matmuls bridge PE to first data

_compiled = {}


def _build():
    import concourse.bass as bass
    import concourse.bacc as bacc
    import concourse.tile as tile
    from concourse import mybir

    F32 = mybir.dt.float32
    BF16 = mybir.dt.bfloat16
    F8 = mybir.dt.float8e4
    AF = mybir.ActivationFunctionType
    ALU = mybir.AluOpType
    DR = mybir.MatmulPerfMode.DoubleRow

    nc = bacc.Bacc(None, target_bir_lowering=False)

    # host layouts (see kernel()):
    #  xb [128, SPC, HC, L] bf16 : bf16(X^T[128j+p, 512s+l])
    #  wbig [128, CW + HC*768] bf16:
    #    cols 0:6   = b_fc[ord[128t+p]]
    #    cols 6:12  = 64*W_nu[ord[128t+p]]
    #    cols 12:44 = fp8 bytes [u, i, 16]: byte 0 of each 16B plane
    #                 holds fp8(64*W_nu[ord[128*(2u+i)+p]]), rest zero
    #                 (dual-fp8 ldweights needs >=8B segments and >=16B
    #                 plane stride)
    #    then t-major weights:
    #    wbig[p, CW + t*768 + hc*128 + m] = WT[128hc+p, ord[128t+m]]
    xb_d = nc.dram_tensor("xb", [128, SPC, HC, L], BF16, kind="ExternalInput")
    wbig_d = nc.dram_tensor("wbig", [128, CW + HC * H], BF16,
                            kind="ExternalInput")
    # unnormalized pooled output stays partition-major; host divides by
    # Z and reshapes to [SPC, H]
    out_d = nc.dram_tensor("out", [128, SPC, HC], F32, kind="ExternalOutput")
    z_d = nc.dram_tensor("zz", [1, SPC], F32, kind="ExternalOutput")

    with tile.TileContext(nc) as tc:
        with tc.tile_pool(name="xp", bufs=1) as xp, \
             tc.tile_pool(name="wp", bufs=1) as wp, \
             tc.tile_pool(name="cst", bufs=1) as cst, \
             tc.tile_pool(name="ups", bufs=3) as upsp, \
             tc.tile_pool(name="sm", bufs=6) as smp, \
             tc.tile_pool(name="mmps", bufs=7, space="PSUM") as mmps, \
             tc.tile_pool(name="nups", bufs=1, space="PSUM") as nups:

            # ---- PE warmup: junk matmuls with no DMA dependency keep the
            # PE pstate ramping while wbig + sample 0 stream in.
            wu_sb = cst.tile([128, 512], BF16)
            nc.vector.memset(wu_sb[:], 1.0)
            wu_ps = mmps.tile([128, 512], F32, tag="mm", name="wu_ps")
            for i in range(WARMUP_MM):
                nc.tensor.matmul(wu_ps[:], wu_sb[:, 0:128], wu_sb[:],
                                 start=(i == 0), stop=(i == WARMUP_MM - 1))

            wbig_sb = wp.tile([128, CW + HC * H], BF16, name="wbig")
            xb_sb = xp.tile([128, SPC, HC, L], BF16, name="xb")
            pall = cst.tile([128, SPC, HC], F32, name="pall")
            zall = cst.tile([1, SPC], F32, name="zall")

            # one big-line transfer for all weights/consts on the sync
            # queue. Transfers sharing a queue interleave (round-robin by
            # descriptor), so the startup-critical samples each get a
            # queue to themselves; the rest are issued in pairs from
            # later tail blocks (see the sample loop).
            WH = CW + 3 * H
            nc.sync.dma_start(wbig_sb[:, 0:WH], wbig_d[:, 0:WH])
            nc.gpsimd.dma_start(xb_sb[:, 0, 0:3], xb_d[:, 0, 0:3])
            nc.scalar.dma_start(xb_sb[:, 0, 3:HC], xb_d[:, 0, 3:HC])
            nc.sync.dma_start(wbig_sb[:, WH:], wbig_d[:, WH:])
            nc.scalar.dma_start(xb_sb[:, 1], xb_d[:, 1])
            nc.sync.dma_start(xb_sb[:, 2], xb_d[:, 2])

            # [128, u, i, m] fp8 view of the packed W_nu head: planes
            # are 16B apart, ldweights reads the first 8 of each
            wnu8 = wbig_sb[:, 2 * HC:CW].bitcast(F8).rearrange(
                "p (u i m) -> p u i m", u=N8C // 2, i=2)[:, :, :, 0:NUM]

            def mm_tiles(s, ups8, upsb, trange):
                for t in trange:
                    ps = mmps.tile([128, L], F32, tag="mm")
                    for hc in range(HC):
                        nc.tensor.matmul(
                            ps[:],
                            wbig_sb[:, CW + t * H + hc * 128:
                                    CW + t * H + hc * 128 + 128],
                            xb_sb[:, s, hc, :],
                            start=(hc == 0), stop=(hc == HC - 1),
                        )
                    dst = ups8[:, t, :] if t < N8C else upsb[:, t - N8C, :]
                    nc.scalar.activation(
                        dst, ps[:], AF.Tanh, bias=wbig_sb[:, t:t + 1],
                    )

            def tail_nu(s, ups8, upsb):
                # nu*64: two fp8 DoubleRow passes over the low-|W_nu|
                # chunks + two bf16 passes over the high ones
                nu = nups.tile([NUM, L], F32, tag="nu")
                for u in range(N8C // 2):
                    nc.tensor.matmul(
                        nu[:], wnu8[:, u],
                        ups8[:, 2 * u:2 * u + 2, :],
                        start=(u == 0), stop=False,
                        perf_mode=DR, skip_group_check=True,
                    )
                for k in range(N8C, HC):
                    nc.tensor.matmul(
                        nu[0:1, :], wbig_sb[:, HC + k:HC + k + 1],
                        upsb[:, k - N8C, :],
                        start=False, stop=(k == HC - 1),
                        skip_group_check=True,
                    )
                return nu

            def tail_soft(s, nu, last=False):
                # softmax over the 512 logits (single partition); exp's
                # scale folds away the x64 on W_nu; Z ships to the host
                # (which divides), so no reciprocal/rescale on device.
                # logits are small enough that exp() needs no max
                # subtraction.
                ex = smp.tile([1, L], BF16, tag="ex")
                nc.scalar.activation(ex[:], nu[0:1, :], AF.Exp,
                                     scale=1.0 / 64.0,
                                     accum_out=zall[0:1, s:s + 1])

                # broadcast unnormalized E, pool in bf16 straight into
                # the output tile
                ab = smp.tile([128, L], BF16, tag="ab")
                nc.gpsimd.partition_broadcast(ab[:], ex[:])
                for j in range(HC):
                    trash = smp.tile([128, L], BF16, tag="trash")
                    nc.vector.scalar_tensor_tensor(
                        trash[:],
                        xb_sb[:, s, j, :],
                        1.0,
                        ab[:],
                        ALU.mult,
                        ALU.mult,
                        accum_out=pall[:, s, j:j + 1],
                    )

            deferred = {1: (3, 4), 2: (5, 6), 3: (7,)}
            prev = None      # (s, ups8, upsb)
            for s in range(SPC):
                ups8 = upsp.tile([128, N8C, L], F8, tag="ups8", name="ups8")
                upsb = upsp.tile([128, HC - N8C, L], BF16, tag="upsb")
                mm_tiles(s, ups8, upsb, range(0, 2))
                if prev is not None:
                    pnu = tail_nu(prev[0], prev[1], prev[2])
                mm_tiles(s, ups8, upsb, range(2, HC))
                if prev is not None:
                    tail_soft(prev[0], pnu)
                for sd in deferred.get(s, ()):
                    nc.gpsimd.dma_start(xb_sb[:, sd], xb_d[:, sd])
                prev = (s, ups8, upsb)
            pnu = tail_nu(prev[0], prev[1], prev[2])
            tail_soft(SPC - 1, pnu, last=True)

            # z completes ~5us before the pooled sums; issue it first so
            # only the pall transfer sits on the final-barrier path
            nc.sync.dma_start(z_d[:], zall[:])
            nc.sync.dma_start(out_d[:], pall[:])

    nc.finalize()
    return nc


def _prep_host(hidden_states, W_fc, b_fc, W_nu):
    bf = ml_dtypes.bfloat16
    f8 = ml_dtypes.float8_e4m3fn
    hs = np.ascontiguousarray(hidden_states, dtype=np.float32)
    W_fc = np.asarray(W_fc, np.float32)
    b_fc = np.asarray(b_fc, np.float32)
    W_nu = np.asarray(W_nu, np.float32)

    order = np.argsort(np.abs(W_nu), kind="stable")
    WT = W_fc.T[:, order]                                # [hin, kout sorted]
    wnu64 = (W_nu[order] * 64.0).reshape(HC, 128)        # [t, p]

    wbig = np.empty((128, CW + HC * H), dtype=bf)
    wbig[:, 0:HC] = b_fc[order].reshape(HC, 128).T.astype(bf)
    wbig[:, HC:2 * HC] = wnu64.T.astype(bf)
    # packed fp8 wnu for the DoubleRow nu: 16B [u, i] planes, wnu at m=0
    pk = wnu64[0:N8C].astype(f8).view(np.uint8)          # [4 chunks, 128]
    head = np.zeros((128, N8C, 16), np.uint8)
    head[:, :, 0] = pk.T
    wbig[:, 2 * HC:CW].view(np.uint8)[:] = head.reshape(128, N8C * 16)
    w = WT.reshape(HC, 128, HC, 128).transpose(1, 2, 0, 3)  # [p, t, hc, m]
    wbig[:, CW:] = np.ascontiguousarray(w).reshape(128, HC * H).astype(bf)

    xbs = []
    for c in range(NCORES):
        xt = hs[c * SPC:(c + 1) * SPC].reshape(SPC * L, H).T  # [H, TOK]
        v = xt.reshape(HC, 128, SPC, L).transpose(1, 2, 0, 3)  # [p,s,j,l]
        xbs.append(np.ascontiguousarray(v).astype(bf))
    return wbig, xbs


def kernel(hidden_states, W_fc, b_fc, W_nu, _trace=False, _trace_kwargs=None):
    from concourse.bass_utils import run_bass_kernel_spmd

    wbig, xbs = _prep_host(hidden_states, W_fc, b_fc, W_nu)
    in_maps = [{"xb": xbs[c], "wbig": wbig} for c in range(NCORES)]

    if "nc" not in _compiled:
        _compiled["nc"] = _build()
    res = run_bass_kernel_spmd(
        _compiled["nc"], in_maps, list(range(NCORES)),
        trace=_trace, **(_trace_kwargs or {}),
    )
    kernel.last_results = res
    outs = []
    for r in res.results:
        p = np.asarray(r["out"], np.float32).transpose(1, 2, 0).reshape(SPC, H)
        z = np.asarray(r["zz"], np.float32).reshape(SPC, 1)
        outs.append(p / z)
    return np.concatenate(outs).astype(np.float32)


# revision 27
# speedup vs baseline: 1.0198x; 1.0198x over previous
"""AttentionProtoNet pooling kernel for 8x TRN2 NeuronCores.

reference (per sample of B=64, L=512, H=768):
    upsilon = tanh(hs @ W_fc.T + b_fc)        [L, H]
    nu      = upsilon @ W_nu                  [L]
    alphas  = softmax(nu)                     [L]
    pooled  = alphas @ hs                     [H]

Strategy: data-parallel over B (8 samples per core). The big GEMM runs in
bf16 (1 cycle/row on the PE at the full 2.4 GHz pstate) against a single
bf16 X^T copy that also feeds the pooling stage. Output channels are
sorted by |W_nu| on the host: upsilon only exists to produce the scalar
nu = W_nu . tanh(...), so the 512 lowest-|W_nu| channels can round their
tanh output to fp8e4 with negligible effect, letting the nu contraction
run as two fp8 DoubleRow matmuls (256-deep, 0.5 cyc/row) plus two bf16
ones - half the PE cost of a pure bf16 nu. W_nu rides along x64 (fp8
needs the scale to stay normal; exp() folds 1/64 back in for free).
Weights + biases + W_nu ship as ONE contiguous-per-partition DMA (small
strided lines run at ~30 GB/s vs ~170 GB/s for 8KB lines); X samples are
monolithic per-sample transfers split across the gpsimd/scalar queues.
tanh runs on ACT straight out of PSUM (per-partition bias), softmax on 1
partition with bf16 exp, alphas broadcast via GpSimd, weighted-sum
pooling on the VectorEngine in bf16, outputs drain per-sample through a
tiny PE transpose. Sample s's tail (nu/softmax/pool) is interleaved into
sample s+1's GEMM block so the PE stream stays dense.
"""

import sys

sys.path.insert(0, "/opt/trn_rl_repo")

import numpy as np
import ml_dtypes

B, L, H = 64, 512, 768
NCORES = 8
SPC = B // NCORES            # samples per core
HC = H // 128                # 128-partition chunks of H
N8C = 4                      # ups chunks (lowest |W_nu|) rounded to fp8
NUM = 8                      # wnu occupies m=0 of each 16B-strided plane
CW = HC + HC + N8C * 16 // 2   # head cols: bias | wnu*64 bf16 | packed fp8
WARMUP_MM = 22               # junk matmuls bridge PE to first data

_compiled = {}


def _build():
    import concourse.bass as bass
    import concourse.bacc as bacc
    import concourse.tile as tile
    from concourse import mybir

    F32 = mybir.dt.float32
    BF16 = mybir.dt.bfloat16
    F8 = mybir.dt.float8e4
    AF = mybir.ActivationFunctionType
    ALU = mybir.AluOpType
    DR = mybir.MatmulPerfMode.DoubleRow

    nc = bacc.Bacc(None, target_bir_lowering=False)

    # host layouts (see kernel()):
    #  xb [128, SPC, HC, L] bf16 : bf16(X^T[128j+p, 512s+l])
    #  wbig [128, CW + HC*768] bf16:
    #    cols 0:6   = b_fc[ord[128t+p]]
    #    cols 6:12  = 64*W_nu[ord[128t+p]]
    #    cols 12:44 = fp8 bytes [u, i, 16]: byte 0 of each 16B plane
    #                 holds fp8(64*W_nu[ord[128*(2u+i)+p]]), rest zero
    #                 (dual-fp8 ldweights needs >=8B segments and >=16B
    #                 plane stride)
    #    then t-major weights:
    #    wbig[p, CW + t*768 + hc*128 + m] = WT[128hc+p, ord[128t+m]]
    xb_d = nc.dram_tensor("xb", [128, SPC, HC, L], BF16, kind="ExternalInput")
    wbig_d = nc.dram_tensor("wbig", [128, CW + HC * H], BF16,
                            kind="ExternalInput")
    # unnormalized pooled output stays partition-major; host divides by
    # Z and reshapes to [SPC, H]
    out_d = nc.dram_tensor("out", [128, SPC, HC], F32, kind="ExternalOutput")
    z_d = nc.dram_tensor("zz", [1, SPC], F32, kind="ExternalOutput")

    with tile.TileContext(nc) as tc:
        with tc.tile_pool(name="xp", bufs=1) as xp, \
             tc.tile_pool(name="wp", bufs=1) as wp, \
             tc.tile_pool(name="cst", bufs=1) as cst, \
             tc.tile_pool(name="ups", bufs=3) as upsp, \
             tc.tile_pool(name="sm", bufs=6) as smp, \
             tc.tile_pool(name="mmps", bufs=7, space="PSUM") as mmps, \
             tc.tile_pool(name="nups", bufs=1, space="PSUM") as nups:

            # ---- PE warmup: junk matmuls with no DMA dependency keep the
            # PE pstate ramping while wbig + sample 0 stream in.
            wu_sb = cst.tile([128, 512], BF16)
            nc.vector.memset(wu_sb[:], 1.0)
            wu_ps = mmps.tile([128, 512], F32, tag="mm", name="wu_ps")
            for i in range(WARMUP_MM):
                nc.tensor.matmul(wu_ps[:], wu_sb[:, 0:128], wu_sb[:],
                                 start=(i == 0), stop=(i == WARMUP_MM - 1))

            wbig_sb = wp.tile([128, CW + HC * H], BF16, name="wbig")
            xb_sb = xp.tile([128, SPC, HC, L], BF16, name="xb")
            pall = cst.tile([128, SPC, HC], F32, name="pall")
            zall = cst.tile([1, SPC], F32, name="zall")

            # one big-line transfer for all weights/consts on the sync
            # queue. Transfers sharing a queue interleave (round-robin by
            # descriptor), so the startup-critical samples each get a
            # queue to themselves; the rest are issued in pairs from
            # later tail blocks (see the sample loop).
            WH = CW + 3 * H
            nc.sync.dma_start(wbig_sb[:, 0:WH], wbig_d[:, 0:WH])
            nc.gpsimd.dma_start(xb_sb[:, 0, 0:3], xb_d[:, 0, 0:3])
            nc.scalar.dma_start(xb_sb[:, 0, 3:HC], xb_d[:, 0, 3:HC])
            nc.sync.dma_start(wbig_sb[:, WH:], wbig_d[:, WH:])
            nc.scalar.dma_start(xb_sb[:, 1], xb_d[:, 1])
            nc.sync.dma_start(xb_sb[:, 2], xb_d[:, 2])

            # [128, u, i, m] fp8 view of the packed W_nu head: planes
            # are 16B apart, ldweights reads the first 8 of each
            wnu8 = wbig_sb[:, 2 * HC:CW].bitcast(F8).rearrange(
                "p (u i m) -> p u i m", u=N8C // 2, i=2)[:, :, :, 0:NUM]

            def mm_tiles(s, ups8, upsb, trange):
                for t in trange:
                    ps = mmps.tile([128, L], F32, tag="mm")
                    for hc in range(HC):
                        nc.tensor.matmul(
                            ps[:],
                            wbig_sb[:, CW + t * H + hc * 128:
                                    CW + t * H + hc * 128 + 128],
                            xb_sb[:, s, hc, :],
                            start=(hc == 0), stop=(hc == HC - 1),
                        )
                    dst = ups8[:, t, :] if t < N8C else upsb[:, t - N8C, :]
                    nc.scalar.activation(
                        dst, ps[:], AF.Tanh, bias=wbig_sb[:, t:t + 1],
                    )

            def tail_nu(s, ups8, upsb):
                # nu*64: two fp8 DoubleRow passes over the low-|W_nu|
                # chunks + two bf16 passes over the high ones
                nu = nups.tile([NUM, L], F32, tag="nu")
                for u in range(N8C // 2):
                    nc.tensor.matmul(
                        nu[:], wnu8[:, u],
                        ups8[:, 2 * u:2 * u + 2, :],
                        start=(u == 0), stop=False,
                        perf_mode=DR, skip_group_check=True,
                    )
                for k in range(N8C, HC):
                    nc.tensor.matmul(
                        nu[0:1, :], wbig_sb[:, HC + k:HC + k + 1],
                        upsb[:, k - N8C, :],
                        start=False, stop=(k == HC - 1),
                        skip_group_check=True,
                    )
                return nu

            def tail_soft(s, nu, last=False):
                # softmax over the 512 logits (single partition); exp's
                # scale folds away the x64 on W_nu; Z ships to the host
                # (which divides), so no reciprocal/rescale on device.
                # logits are small enough that exp() needs no max
                # subtraction.
                ex = smp.tile([1, L], BF16, tag="ex")
                nc.scalar.activation(ex[:], nu[0:1, :], AF.Exp,
                                     scale=1.0 / 64.0,
                                     accum_out=zall[0:1, s:s + 1])

                # broadcast unnormalized E, pool in bf16 straight into
                # the output tile
                ab = smp.tile([128, L], BF16, tag="ab")
                nc.gpsimd.partition_broadcast(ab[:], ex[:])
                if last:
                    # the last sample's pool is the exposed tail: fan the
                    # reduction across DVE (product) + ACT (free-axis sum)
                    # for half the chunks so the two engines overlap
                    prods = []
                    for j in range(HC // 2):
                        prod = smp.tile([128, L], BF16, tag="trash")
                        nc.vector.tensor_tensor(
                            prod[:], xb_sb[:, s, j, :], ab[:], ALU.mult)
                        prods.append(prod)
                    for j, prod in enumerate(prods):
                        sink = smp.tile([128, L], BF16, tag="sink")
                        nc.scalar.activation(
                            sink[:], prod[:], AF.Copy,
                            accum_out=pall[:, s, j:j + 1])
                rng = range(HC // 2, HC) if last else range(HC)
                for j in rng:
                    trash = smp.tile([128, L], BF16, tag="trash")
                    nc.vector.scalar_tensor_tensor(
                        trash[:],
                        xb_sb[:, s, j, :],
                        1.0,
                        ab[:],
                        ALU.mult,
                        ALU.mult,
                        accum_out=pall[:, s, j:j + 1],
                    )

            deferred = {1: (3, 4), 2: (5, 6), 3: (7,)}
            prev = None      # (s, ups8, upsb)
            for s in range(SPC):
                ups8 = upsp.tile([128, N8C, L], F8, tag="ups8", name="ups8")
                upsb = upsp.tile([128, HC - N8C, L], BF16, tag="upsb")
                mm_tiles(s, ups8, upsb, range(0, 2))
                if prev is not None:
                    pnu = tail_nu(prev[0], prev[1], prev[2])
                mm_tiles(s, ups8, upsb, range(2, HC))
                if prev is not None:
                    tail_soft(prev[0], pnu)
                for sd in deferred.get(s, ()):
                    nc.gpsimd.dma_start(xb_sb[:, sd], xb_d[:, sd])
                prev = (s, ups8, upsb)
            pnu = tail_nu(prev[0], prev[1], prev[2])
            tail_soft(SPC - 1, pnu, last=True)

            # z completes ~5us before the pooled sums; issue it first so
            # only the pall transfer sits on the final-barrier path
            nc.sync.dma_start(z_d[:], zall[:])
            nc.sync.dma_start(out_d[:], pall[:])

    nc.finalize()
    return nc


def _prep_host(hidden_states, W_fc, b_fc, W_nu):
    bf = ml_dtypes.bfloat16
    f8 = ml_dtypes.float8_e4m3fn
    hs = np.ascontiguousarray(hidden_states, dtype=np.float32)
    W_fc = np.asarray(W_fc, np.float32)
    b_fc = np.asarray(b_fc, np.float32)
    W_nu = np.asarray(W_nu, np.float32)

    order = np.argsort(np.abs(W_nu), kind="stable")
    WT = W_fc.T[:, order]                                # [hin, kout sorted]
    wnu64 = (W_nu[order] * 64.0).reshape(HC, 128)        # [t, p]

    wbig = np.empty((128, CW + HC * H), dtype=bf)
    wbig[:, 0:HC] = b_fc[order].reshape(HC, 128).T.astype(bf)
    wbig[:, HC:2 * HC] = wnu64.T.astype(bf)
    # packed fp8 wnu for the DoubleRow nu: 16B [u, i] planes, wnu at m=0
    pk = wnu64[0:N8C].astype(f8).view(np.uint8)          # [4 chunks, 128]
    head = np.zeros((128, N8C, 16), np.uint8)
    head[:, :, 0] = pk.T
    wbig[:, 2 * HC:CW].view(np.uint8)[:] = head.reshape(128, N8C * 16)
    w = WT.reshape(HC, 128, HC, 128).transpose(1, 2, 0, 3)  # [p, t, hc, m]
    wbig[:, CW:] = np.ascontiguousarray(w).reshape(128, HC * H).astype(bf)

    xbs = []
    for c in range(NCORES):
        xt = hs[c * SPC:(c + 1) * SPC].reshape(SPC * L, H).T  # [H, TOK]
        v = xt.reshape(HC, 128, SPC, L).transpose(1, 2, 0, 3)  # [p,s,j,l]
        xbs.append(np.ascontiguousarray(v).astype(bf))
    return wbig, xbs


def kernel(hidden_states, W_fc, b_fc, W_nu, _trace=False, _trace_kwargs=None):
    from concourse.bass_utils import run_bass_kernel_spmd

    wbig, xbs = _prep_host(hidden_states, W_fc, b_fc, W_nu)
    in_maps = [{"xb": xbs[c], "wbig": wbig} for c in range(NCORES)]

    if "nc" not in _compiled:
        _compiled["nc"] = _build()
    res = run_bass_kernel_spmd(
        _compiled["nc"], in_maps, list(range(NCORES)),
        trace=_trace, **(_trace_kwargs or {}),
    )
    kernel.last_results = res
    outs = []
    for r in res.results:
        p = np.asarray(r["out"], np.float32).transpose(1, 2, 0).reshape(SPC, H)
        z = np.asarray(r["zz"], np.float32).reshape(SPC, 1)
        outs.append(p / z)
    return np.concatenate(outs).astype(np.float32)
